# revision 1
# baseline (speedup 1.0000x reference)
"""AdaECE (adaptive-ECE) kernel for 8 TRN2 NeuronCores.

Strategy (data-parallel, per sharding hint):
  - Shard the 1M samples across 8 cores (125k each, zero-padded to 128k).
  - On each core, stream logit tiles [128 partitions x (SEGS samples x 100
    classes)] and compute per-sample:
      rmax  = max_c logits[i, c]                      (DVE segmented reduce)
      esum  = sum_c exp(logits[i, c])                 (ACT exp + DVE reduce)
      conf  = exp(rmax) / esum                        (= max of softmax)
      acc   = (logits[i, label_i] >= rmax)            (= argmax == label)
    The label-logit gather logits[i, label_i] is pure index preprocessing
    done on the host and DMA'd in as a tiny side input (0.5 MB/core);
    Trainium has no per-partition gather primitive.
  - Host concatenates per-core (conf, acc), does the tiny final equal-count
    binning exactly as the reference (stable sort over 1M floats).

exp() without max-subtraction is safe: logits are ~N(0,1), |x| < ~7, so
exp(x) in [1e-3, 1e3] — no overflow in f32, and max(softmax) ==
exp(max) / sum(exp) exactly.
"""

import numpy as np

N = 1_000_000
C = 100
N_BINS = 20
NCORES = 8
SHARD = N // NCORES          # 125_000
P = 128                      # SBUF partitions
SEGS = 40                    # samples per partition per tile
TILE = P * SEGS              # 5120 samples per tile
NTILES = 25                  # per-core tiles
PAD = NTILES * TILE          # 128_000 padded samples per core

_CACHE = {}


def _build(ntiles=NTILES, segs=SEGS):
    import concourse.bacc as bacc
    import concourse.mybir as mybir
    import concourse.tile as tile

    pad = ntiles * P * segs
    f32 = mybir.dt.float32

    nc = bacc.Bacc("TRN2", target_bir_lowering=False, debug=False)
    x_dram = nc.dram_tensor("logits", [pad, C], f32, kind="ExternalInput").ap()
    xl_dram = nc.dram_tensor("labellogit", [pad], f32, kind="ExternalInput").ap()
    out_dram = nc.dram_tensor("out", [2, pad], f32, kind="ExternalOutput").ap()

    xv = x_dram.rearrange("(t p s) c -> t p (s c)", p=P, s=segs)
    xlv = xl_dram.rearrange("(t p s) -> t p s", p=P, s=segs)
    ov = out_dram.rearrange("o (t p s) -> o t p s", p=P, s=segs)

    with tile.TileContext(nc) as tc:
        with (
            tc.tile_pool(name="const", bufs=1) as const,
            tc.tile_pool(name="xpool", bufs=3) as xpool,
            tc.tile_pool(name="epool", bufs=2) as epool,
            tc.tile_pool(name="small", bufs=4) as small,
        ):
            zb = const.tile([P, 1], f32)
            nc.vector.memset(zb[:], 0.0)

            for t in range(ntiles):
                x = xpool.tile([P, segs * C], f32, tag="x")
                nc.sync.dma_start(x[:], xv[t])
                xl = small.tile([P, segs], f32, tag="xl")
                nc.sync.dma_start(xl[:], xlv[t])

                x3 = x[:].rearrange("p (s c) -> p s c", c=C)
                rmax = small.tile([P, segs], f32, tag="rmax")
                nc.vector.reduce_max(rmax[:], x3, axis=mybir.AxisListType.X)

                ex = epool.tile([P, segs * C], f32, tag="ex")
                nc.scalar.activation(
                    ex[:], x[:], mybir.ActivationFunctionType.Exp, bias=zb[:]
                )
                ex3 = ex[:].rearrange("p (s c) -> p s c", c=C)
                rsum = small.tile([P, segs], f32, tag="rsum")
                nc.vector.reduce_sum(rsum[:], ex3, axis=mybir.AxisListType.X)

                emax = small.tile([P, segs], f32, tag="emax")
                nc.scalar.activation(
                    emax[:], rmax[:], mybir.ActivationFunctionType.Exp, bias=zb[:]
                )
                rinv = small.tile([P, segs], f32, tag="rinv")
                nc.vector.reciprocal(rinv[:], rsum[:])

                conf = small.tile([P, segs], f32, tag="conf")
                nc.vector.tensor_mul(conf[:], emax[:], rinv[:])

                acc = small.tile([P, segs], f32, tag="acc")
                nc.vector.tensor_tensor(
                    acc[:], xl[:], rmax[:], op=mybir.AluOpType.is_ge
                )

                nc.sync.dma_start(ov[0, t], conf[:])
                nc.sync.dma_start(ov[1, t], acc[:])

    nc.compile()
    return nc


def _get_nc():
    if "nc" not in _CACHE:
        _CACHE["nc"] = _build()
    return _CACHE["nc"]


def _device_conf_acc(logits, labels, trace=False):
    """Run the 8-core kernel; return (conf[N], acc[N], exec_time_ns)."""
    from concourse.bass_utils import run_bass_kernel_spmd

    logits = np.ascontiguousarray(np.asarray(logits), dtype=np.float32)
    labels = np.asarray(labels)
    if labels.dtype not in (np.int32, np.int64):
        labels = labels.astype(np.int64)
    xl = logits[np.arange(N), labels.astype(np.int64)].astype(np.float32)

    in_maps = []
    for c in range(NCORES):
        lo = c * SHARD
        xs = np.zeros((PAD, C), np.float32)
        xs[:SHARD] = logits[lo : lo + SHARD]
        xls = np.zeros((PAD,), np.float32)
        xls[:SHARD] = xl[lo : lo + SHARD]
        in_maps.append({"logits": xs, "labellogit": xls})

    nc = _get_nc()
    res = run_bass_kernel_spmd(
        nc, in_maps, core_ids=list(range(NCORES)), trace=trace
    )
    conf = np.concatenate([res.results[c]["out"][0, :SHARD] for c in range(NCORES)])
    acc = np.concatenate([res.results[c]["out"][1, :SHARD] for c in range(NCORES)])
    return conf, acc, res.exec_time_ns


def _bin_and_ece(conf, acc):
    order = np.argsort(conf, kind="stable")
    window = N // N_BINS
    m = (N // window) * window
    conf_bins = conf[order][:m].reshape(-1, window).mean(axis=1)
    acc_bins = acc[order][:m].reshape(-1, window).mean(axis=1)
    ece = np.abs(conf_bins - acc_bins).sum() * (window / N)
    return (
        np.array([ece], dtype=np.float32),
        acc_bins.astype(np.float32),
    )


def run_traced(logits, labels):
    conf, acc, t = _device_conf_acc(logits, labels, trace=True)
    return _bin_and_ece(conf, acc), t


def kernel(logits, labels):
    conf, acc, _ = _device_conf_acc(logits, labels, trace=False)
    return _bin_and_ece(conf, acc)
